# revision 7
# baseline (speedup 1.0000x reference)
"""MoE expert-network kernel for 8 Trainium2 NeuronCores.

Strategy: expert parallelism (E == n_cores == 8). The host dispatches each
token to its expert's core (an all-to-all in numpy), folds the inference-mode
BatchNorm into the expert weights/bias, and each core runs one dense
[cap, 512] @ [512, 512] GEMM fused with bias + SiLU via the activation engine.

Per-core device program (identical on all cores, SPMD):
  inputs : xT [512, cap]  - this expert's tokens, transposed (IN on rows)
           w  [512, 512]  - BN-folded weight, [IN, HID] layout
           b  [512, 1]    - BN-folded bias
  output : o  [512, cap]  - silu(w.T @ xT + b), HID on rows
The host scatters o back into the full [B, 512] output.
"""

import sys

for _p in ("/opt/trn_rl_repo",):
    if _p not in sys.path:
        sys.path.append(_p)

import numpy as np

import concourse.bass as bass
import concourse.mybir as mybir
import concourse.tile as tile
from concourse import bacc
from concourse.bass_utils import run_bass_kernel_spmd

B = 32768
IN = 512
HID = 512
E = 8
NCORES = 8
EPS = 1e-5
P = 128  # SBUF partitions
NT = 512  # token tile (matmul moving free dim; one PSUM bank at fp32)

KC = IN // P  # contraction chunks
MC = HID // P  # output-feature chunks


def build_bass(cap: int, act: str = "silu") -> bass.Bass:
    nc = bacc.Bacc(
        "TRN2",
        target_bir_lowering=False,
        debug=False,
        num_devices=NCORES,
    )
    f32 = mybir.dt.float32
    f32r = mybir.dt.float32r

    xT = nc.dram_tensor("xT", [IN, cap], f32, kind="ExternalInput").ap()
    w = nc.dram_tensor("w", [IN, HID], f32, kind="ExternalInput").ap()
    b = nc.dram_tensor("b", [HID, 1], f32, kind="ExternalInput").ap()
    o = nc.dram_tensor("o", [HID, cap], f32, kind="ExternalOutput").ap()

    # [IN, cap] viewed as [P partitions, KC chunks, cap]
    xT_r = xT.rearrange("(k p) n -> p k n", p=P)

    with tile.TileContext(nc) as tc:
        with (
            tc.tile_pool(name="wpool", bufs=1) as wpool,
            tc.tile_pool(name="bpool", bufs=1) as bpool,
            tc.tile_pool(name="xpool", bufs=3) as xpool,
            tc.tile_pool(name="opool", bufs=6) as opool,
            tc.tile_pool(name="pp", bufs=8, space="PSUM") as pp,
        ):
            w_sb = []
            for k in range(KC):
                # float32r: same bits as fp32; fp32r-typed SBUF satisfies the
                # BIR verifier's "consumed by FP32r matmult" producer rule and
                # runs the PE at 1 cycle/row (vs 4 for strict fp32).
                wt = wpool.tile([P, HID], f32r, tag=f"w{k}", name=f"w{k}")
                nc.sync.dma_start(
                    out=wt, in_=w[k * P : (k + 1) * P, :].bitcast(f32r)
                )
                w_sb.append(wt)
            b_sb = []
            for m in range(MC):
                bt = bpool.tile([P, 1], f32, tag=f"b{m}", name=f"b{m}")
                nc.sync.dma_start(out=bt, in_=b[m * P : (m + 1) * P, :])
                b_sb.append(bt)

            n0 = 0
            while n0 < cap:
                nt = min(NT, cap - n0)
                xt = xpool.tile([P, KC, nt], f32r, tag="xt", name="xt")
                nc.sync.dma_start(out=xt, in_=xT_r[:, :, n0 : n0 + nt].bitcast(f32r))
                for m in range(MC):
                    ps = pp.tile([P, nt], f32, tag="ps", name="ps")
                    for k in range(KC):
                        nc.tensor.matmul(
                            ps,
                            lhsT=w_sb[k][:, m * P : (m + 1) * P],
                            rhs=xt[:, k, :],
                            start=(k == 0),
                            stop=(k == KC - 1),
                        )
                    ot = opool.tile([P, nt], f32, tag="ot", name="ot")
                    if act == "silu":
                        nc.scalar.activation(
                            ot, ps, mybir.ActivationFunctionType.Silu, bias=b_sb[m]
                        )
                    else:
                        # CoreSim has no Silu: y*sigmoid(y) from Identity+Sigmoid+mul
                        yt = opool.tile([P, nt], f32, tag="yt", name="yt")
                        nc.scalar.activation(
                            yt, ps, mybir.ActivationFunctionType.Identity, bias=b_sb[m]
                        )
                        st = opool.tile([P, nt], f32, tag="st", name="st")
                        nc.scalar.activation(
                            st, ps, mybir.ActivationFunctionType.Sigmoid, bias=b_sb[m]
                        )
                        nc.vector.tensor_mul(ot, yt, st)
                    nc.sync.dma_start(
                        out=o[m * P : (m + 1) * P, n0 : n0 + nt], in_=ot
                    )
                n0 += nt

    nc.compile()
    return nc


def prepare(inputs: dict) -> tuple:
    x = np.ascontiguousarray(np.asarray(inputs["x"], dtype=np.float32))
    idx = np.asarray(inputs["expert_indices"]).astype(np.int64)
    ew = np.asarray(inputs["expert_weights"], dtype=np.float32)
    eb = np.asarray(inputs["expert_biases"], dtype=np.float32)
    gw = np.asarray(inputs["bn_weights"], dtype=np.float32)
    gb = np.asarray(inputs["bn_biases"], dtype=np.float32)
    rm = np.asarray(inputs["running_mean"], dtype=np.float32)
    rv = np.asarray(inputs["running_var"], dtype=np.float32)

    # Fold inference BN into the expert weight/bias:
    #   y = (x @ W + eb - rm) * gw/sqrt(rv+eps) + gb = x @ (W*s) + (eb-rm)*s + gb
    s = gw / np.sqrt(rv + EPS)
    wf = ew * s[:, None, :]
    bf = (eb - rm) * s + gb

    perms = [np.nonzero(idx == e)[0] for e in range(E)]
    counts = [len(p) for p in perms]
    cap = max(512, -(-max(counts) // P) * P)

    in_maps = []
    for e in range(E):
        xT = np.zeros((IN, cap), dtype=np.float32)
        if counts[e]:
            xT[:, : counts[e]] = x[perms[e]].T
        in_maps.append(
            {
                "xT": xT,
                "w": np.ascontiguousarray(wf[e]),
                "b": np.ascontiguousarray(bf[e].reshape(HID, 1)),
            }
        )
    return cap, perms, counts, in_maps


def combine(results: list, perms, counts) -> np.ndarray:
    out = np.empty((B, HID), dtype=np.float32)
    for e in range(E):
        if counts[e]:
            out[perms[e]] = results[e]["o"][:, : counts[e]].T
    return out


def kernel(**inputs) -> np.ndarray:
    cap, perms, counts, in_maps = prepare(inputs)
    nc = build_bass(cap)
    res = run_bass_kernel_spmd(nc, in_maps, core_ids=list(range(NCORES)))
    return combine(res.results, perms, counts)


# revision 8
# speedup vs baseline: 1.2636x; 1.2636x over previous
"""MoE expert-network kernel for 8 Trainium2 NeuronCores.

Strategy: expert parallelism (E == n_cores == 8). The host dispatches each
token to its expert's core (an all-to-all in numpy), folds the inference-mode
BatchNorm into the expert weights/bias, and each core runs one dense
[cap, 512] @ [512, 512] GEMM fused with bias + SiLU via the activation engine.

Per-core device program (identical on all cores, SPMD):
  inputs : xT [512, cap]  - this expert's tokens, transposed (IN on rows)
           w  [512, 512]  - BN-folded weight, [IN, HID] layout
           b  [512, 1]    - BN-folded bias
  output : o  [512, cap]  - silu(w.T @ xT + b), HID on rows
The host scatters o back into the full [B, 512] output.
"""

import sys

for _p in ("/opt/trn_rl_repo",):
    if _p not in sys.path:
        sys.path.append(_p)

import numpy as np

import concourse.bass as bass
import concourse.mybir as mybir
import concourse.tile as tile
from concourse import bacc
from concourse.bass_utils import run_bass_kernel_spmd

B = 32768
IN = 512
HID = 512
E = 8
NCORES = 8
EPS = 1e-5
P = 128  # SBUF partitions
NT = 512  # token tile (matmul moving free dim; one PSUM bank at fp32)

KC = IN // P  # contraction chunks
MC = HID // P  # output-feature chunks


def build_bass(cap: int, act: str = "silu") -> bass.Bass:
    nc = bacc.Bacc(
        "TRN2",
        target_bir_lowering=False,
        debug=False,
        num_devices=NCORES,
    )
    f32 = mybir.dt.float32
    f32r = mybir.dt.float32r

    xT = nc.dram_tensor("xT", [IN, cap], f32, kind="ExternalInput").ap()
    w = nc.dram_tensor("w", [IN, HID], f32, kind="ExternalInput").ap()
    b = nc.dram_tensor("b", [HID, 1], f32, kind="ExternalInput").ap()
    o = nc.dram_tensor("o", [HID, cap], f32, kind="ExternalOutput").ap()

    # DRAM views with the 128-partition dim first
    xT_r = xT.rearrange("(k p) n -> p k n", p=P).bitcast(f32r)
    w_r = w.rearrange("(k p) h -> p k h", p=P).bitcast(f32r)
    b_r = b.rearrange("(m p) o -> p m o", p=P)
    o_r = o.rearrange("(m p) n -> p m n", p=P)

    # Token tiles: small first tile for fast pipeline ramp, then 1024-wide
    # tiles (4KB contiguous DMA lines).
    tiles = []
    n0 = 0
    while n0 < cap:
        nt = min(512 if n0 == 0 else 1024, cap - n0)
        tiles.append((n0, nt))
        n0 += nt

    with tile.TileContext(nc) as tc:
        with (
            tc.tile_pool(name="wpool", bufs=1) as wpool,
            tc.tile_pool(name="xpool", bufs=4) as xpool,
            tc.tile_pool(name="opool", bufs=3) as opool,
            tc.tile_pool(name="pp", bufs=8, space="PSUM") as pp,
        ):
            # float32r: same bits as fp32; fp32r-typed SBUF satisfies the
            # BIR verifier's "consumed by FP32r matmult" producer rule and
            # runs the PE at 1 cycle/row (vs 4 for strict fp32).
            wt = wpool.tile([P, KC, HID], f32r, tag="wt", name="wt")
            nc.sync.dma_start(out=wt, in_=w_r)
            bt = wpool.tile([P, MC, 1], f32, tag="bt", name="bt")
            nc.sync.dma_start(out=bt, in_=b_r)

            for n0, nt in tiles:
                xt = xpool.tile([P, KC, nt], f32r, tag="xt", name="xt")
                nc.sync.dma_start(out=xt, in_=xT_r[:, :, n0 : n0 + nt])
                ot = opool.tile([P, MC, nt], f32, tag="ot", name="ot")
                for off in range(0, nt, NT):
                    ns = min(NT, nt - off)
                    for m in range(MC):
                        ps = pp.tile([P, ns], f32, tag="ps", name="ps")
                        for k in range(KC):
                            nc.tensor.matmul(
                                ps,
                                lhsT=wt[:, k, m * P : (m + 1) * P],
                                rhs=xt[:, k, off : off + ns],
                                start=(k == 0),
                                stop=(k == KC - 1),
                            )
                        osl = ot[:, m, off : off + ns]
                        if act == "silu":
                            nc.scalar.activation(
                                osl,
                                ps,
                                mybir.ActivationFunctionType.Silu,
                                bias=bt[:, m, :],
                            )
                        else:
                            # CoreSim has no Silu: Identity+Sigmoid+mul
                            yt = opool.tile([P, ns], f32, tag="yt", name="yt")
                            nc.scalar.activation(
                                yt,
                                ps,
                                mybir.ActivationFunctionType.Identity,
                                bias=bt[:, m, :],
                            )
                            st = opool.tile([P, ns], f32, tag="st", name="st")
                            nc.scalar.activation(
                                st,
                                ps,
                                mybir.ActivationFunctionType.Sigmoid,
                                bias=bt[:, m, :],
                            )
                            nc.vector.tensor_mul(osl, yt, st)
                # single output DMA per token tile on the second HWDGE ring
                # (ACT sequencer) so output stores never head-of-line-block
                # input loads on the sync ring.
                nc.scalar.dma_start(out=o_r[:, :, n0 : n0 + nt], in_=ot)

    nc.compile()
    return nc


def prepare(inputs: dict) -> tuple:
    x = np.ascontiguousarray(np.asarray(inputs["x"], dtype=np.float32))
    idx = np.asarray(inputs["expert_indices"]).astype(np.int64)
    ew = np.asarray(inputs["expert_weights"], dtype=np.float32)
    eb = np.asarray(inputs["expert_biases"], dtype=np.float32)
    gw = np.asarray(inputs["bn_weights"], dtype=np.float32)
    gb = np.asarray(inputs["bn_biases"], dtype=np.float32)
    rm = np.asarray(inputs["running_mean"], dtype=np.float32)
    rv = np.asarray(inputs["running_var"], dtype=np.float32)

    # Fold inference BN into the expert weight/bias:
    #   y = (x @ W + eb - rm) * gw/sqrt(rv+eps) + gb = x @ (W*s) + (eb-rm)*s + gb
    s = gw / np.sqrt(rv + EPS)
    wf = ew * s[:, None, :]
    bf = (eb - rm) * s + gb

    perms = [np.nonzero(idx == e)[0] for e in range(E)]
    counts = [len(p) for p in perms]
    cap = max(512, -(-max(counts) // P) * P)

    in_maps = []
    for e in range(E):
        xT = np.zeros((IN, cap), dtype=np.float32)
        if counts[e]:
            xT[:, : counts[e]] = x[perms[e]].T
        in_maps.append(
            {
                "xT": xT,
                "w": np.ascontiguousarray(wf[e]),
                "b": np.ascontiguousarray(bf[e].reshape(HID, 1)),
            }
        )
    return cap, perms, counts, in_maps


def combine(results: list, perms, counts) -> np.ndarray:
    out = np.empty((B, HID), dtype=np.float32)
    for e in range(E):
        if counts[e]:
            out[perms[e]] = results[e]["o"][:, : counts[e]].T
    return out


def kernel(**inputs) -> np.ndarray:
    cap, perms, counts, in_maps = prepare(inputs)
    nc = build_bass(cap)
    res = run_bass_kernel_spmd(nc, in_maps, core_ids=list(range(NCORES)))
    return combine(res.results, perms, counts)
